# revision 20
# baseline (speedup 1.0000x reference)
"""Causal single-head attention (B=4, N=4096, d_in=1024, d_inner=512, d_out=1024)
for 8 Trainium2 NeuronCores.

Sharding: core c -> (batch b=c//2, half h=c%2). Each core handles the 4
global 512-row query blocks {2u+h : u=0..3} of batch b (block-interleaved
for causal load balance) and computes the full K/V projection on-chip.
No collectives; causality enters only through a per-core 0/1 mask input,
so the instruction stream is SPMD-uniform.

All matmul inputs are fp16 (fp32r/bf16 stream slower on this part); PSUM
accumulation stays fp32. Layouts are fully transposed (scores [j,i],
attention output [dv,i]) so the kernel contains no transposes. Softmax is
normalized AFTER the output projection: probability tiles are pre-summed
pairwise on the DVE, reduced to l^T [queries-on-partitions, 1] by tiny
free-dim-1 matmuls, and 1/l is applied as a per-partition ScalarE scale
during the y eviction (with the bias added by the DVE) - the lane-serial
DVE reciprocal never touches more than 4 elements per lane.
"""

import sys

if "/opt/trn_rl_repo" not in sys.path:
    sys.path.insert(0, "/opt/trn_rl_repo")

import numpy as np

import concourse.bacc as bacc
import concourse.mybir as mybir
import concourse.tile as tile
from concourse.bass_utils import run_bass_kernel_spmd

P = 128
B, N, DIN, DI, DO = 4, 4096, 1024, 512, 1024
NCORES = 8
NQ = N // 2          # query rows per core (2048)
NU = 4               # query super-blocks of 512 per core
NJS = N // 512       # key strips of 512 (8)
NKB = N // P         # key blocks of 128 (32)
SCALE = float(DI) ** -0.5

F32 = mybir.dt.float32
BF16 = mybir.dt.bfloat16
FP16 = mybir.dt.float16
AF = mybir.ActivationFunctionType

_COMPILED = None


def _build():
    nc = bacc.Bacc(None, target_bir_lowering=False)

    # host-packed layouts: per-partition-contiguous inner blocks
    xt_d = nc.dram_tensor("xt", [P, NJS, 8, 512], FP16, kind="ExternalInput")
    xqt_d = nc.dram_tensor("xqt", [P, NU, 8, 512], FP16, kind="ExternalInput")
    wq_d = nc.dram_tensor("wq", [P, 8, DI], FP16, kind="ExternalInput")
    wk_d = nc.dram_tensor("wk", [P, 8, DI], FP16, kind="ExternalInput")
    wv_d = nc.dram_tensor("wv", [P, 8, DI], FP16, kind="ExternalInput")
    wout_d = nc.dram_tensor("wout", [P, 4, DO], FP16, kind="ExternalInput")
    bout_d = nc.dram_tensor("bout", [P, DO], F32, kind="ExternalInput")
    mask_d = nc.dram_tensor("mask", [P, 8, 512], FP16, kind="ExternalInput")
    y_d = nc.dram_tensor("y", [NQ, DO], F32, kind="ExternalOutput")

    with tile.TileContext(nc) as tc:
        with tc.tile_pool(name="persist", bufs=1) as pp:
            kt = pp.tile([P, 4, N], FP16)        # K^T  [dk, j]   32KB/part
            vt = pp.tile([P, NKB, DI], FP16)     # V    [j, dv]   32KB/part
            wq = pp.tile([P, 8, DI], FP16)
            bout = pp.tile([P, DO], F32)
            ones = pp.tile([P, P], FP16)
            xqs_t = [pp.tile([P, 8, 512], FP16, name=f"xq{u}", tag="xqs", bufs=1)
                     for u in range(NU)]
            qts = [pp.tile([P, 4, 512], FP16, name=f"qt{u}", tag="qt", bufs=2)
                   for u in range(NU)]
            qts0 = qts[0]

            def qproj(u, qt, pool=None, tag="sc", bufs=3):
                """Q^T projection for super-block u into qt [P, 4, 512]."""
                for dq in range(4):
                    qps = (pool or psB).tile([P, 512], F32, name=f"q{u}_{dq}",
                                             tag=tag, bufs=bufs)
                    for c in range(8):
                        nc.tensor.matmul(
                            qps[:], wq[:, c, dq * P:(dq + 1) * P], xqs_t[u][:, c, :],
                            start=(c == 0), stop=(c == 7),
                        )
                    if pool is not None and dq % 2:
                        # u=0 runs at the PSUM pool handoff: alternate the
                        # eviction engine so the last one lands sooner
                        nc.scalar.activation(qt[:, dq, :], qps[:], AF.Copy)
                    else:
                        nc.vector.tensor_copy(qt[:, dq, :], qps[:])

            # ---- Phase A: K^T and V projection over all N keys ----
            # DMA order: first-strip deps land first; phase-B inputs trickle in.
            pa = tc.tile_pool(name="phaseA", bufs=1)
            wp = pa.__enter__()
            psKV_cm = tc.tile_pool(name="psKV", bufs=1, space="PSUM")
            psKV = psKV_cm.__enter__()
            wk = wp.tile([P, 8, DI], FP16)
            wv = wp.tile([P, 8, DI], FP16)
            xs0 = wp.tile([P, 8, 512], FP16, name="xs0", tag="xs", bufs=4)
            # one big DMA per operand half: each dma_start costs ~600ns of
            # queue-engine issue time, so few large transfers beat many small
            nc.sync.dma_start(wk[:, 0:2, :], wk_d.ap()[:, 0:2, :])
            nc.sync.dma_start(wk[:, 2:8, :], wk_d.ap()[:, 2:8, :])
            nc.gpsimd.dma_start(xs0[:, 0:2, :], xt_d.ap()[:, 0, 0:2, :])
            nc.gpsimd.dma_start(xs0[:, 2:8, :], xt_d.ap()[:, 0, 2:8, :])
            nc.gpsimd.dma_start(wv[:], wv_d.ap())
            late_dmas = {
                2: lambda: nc.sync.dma_start(xqs_t[0][:], xqt_d.ap()[:, 0]),
                4: lambda: nc.sync.dma_start(wq[:], wq_d.ap()),
                5: lambda: nc.sync.dma_start(mask_t[:, 0:4, :],
                                             mask_d.ap()[:, 0:4, :]),
                6: lambda: (nc.sync.dma_start(bout[:], bout_d.ap()),
                            nc.sync.dma_start(mask_t[:, 4:8, :],
                                              mask_d.ap()[:, 4:8, :])),
                7: lambda: nc.sync.dma_start(wout_t[:], wout_d.ap()),
            }
            nc.vector.memset(ones[:], 1.0)
            mask_t = pp.tile([P, 8, 512], FP16, name="mask")
            wout_t = pp.tile([P, 4, DO], FP16, name="wout")
            xss = {0: xs0}

            def kproj_strip(js):
                xs = xss[js]
                for dk in range(4):
                    kps = psKV.tile([P, 512], F32, name=f"k{js}_{dk}", tag="kv", bufs=8)
                    for c in range(8):
                        nc.tensor.matmul(
                            kps[:], wk[:, c, dk * P:(dk + 1) * P], xs[:, c, :],
                            start=(c == 0), stop=(c == 7),
                        )
                    nc.vector.tensor_copy(kt[:, dk, js * 512:(js + 1) * 512], kps[:])

            def vproj_strip(js):
                xs = xss[js]
                for jsub in range(4):
                    vps = psKV.tile([P, 512], F32, name=f"v{js}_{jsub}", tag="kv", bufs=8)
                    for c in range(8):
                        nc.tensor.matmul(
                            vps[:], xs[:, c, jsub * P:(jsub + 1) * P], wv[:, c, :],
                            start=(c == 0), stop=(c == 7),
                        )
                    nc.vector.tensor_copy(vt[:, js * 4 + jsub, :], vps[:])

            def load_strip(js):
                xs = wp.tile([P, 8, 512], FP16, name=f"xs{js}", tag="xs", bufs=4)
                nc.sync.dma_start(xs[:, 0:4, :], xt_d.ap()[:, js, 0:4, :])
                nc.gpsimd.dma_start(xs[:, 4:8, :], xt_d.ap()[:, js, 4:8, :])
                xss[js] = xs

            # strips 0-1: both K projections before any V so the head of the
            # DMA stream only carries wk/xs; wv arrives while K runs
            load_strip(1)
            kproj_strip(0)
            late_dmas[2]()
            kproj_strip(1)
            vproj_strip(0)
            vproj_strip(1)
            for js in range(2, NJS):
                load_strip(js)
                if js in late_dmas:
                    late_dmas[js]()
                kproj_strip(js)
                vproj_strip(js)

            qproj(0, qts0, pool=psKV, tag="kv", bufs=8)
            psKV_cm.__exit__(None, None, None)
            pa.__exit__(None, None, None)

            # ---- Phase B: per query super-block u ----
            pb = tc.tile_pool(name="phaseB", bufs=1)
            wp = pb.__enter__()
            psB_cm = tc.tile_pool(name="psB", bufs=1, space="PSUM")
            psB = psB_cm.__enter__()
            mask = mask_t
            wout = wout_t


            LAG = 2
            for u in range(NU):
                nkb = 8 * u + 8  # key blocks (128) this super-block attends to
                ng = nkb // 4    # l-sum groups of 4 key blocks
                qt = qts[u]
                if u + 1 < NU:
                    nc.gpsimd.dma_start(xqs_t[u + 1][:], xqt_d.ap()[:, u + 1])

                outT = [
                    psB.tile([P, 512], F32, name=f"o{u}_{d}", tag=f"outT{d}", bufs=1)
                    for d in range(4)
                ]
                p_ts = {}
                accs = []
                tree = []  # (level, tile) stack for pairwise fp16 l summation
                treen = [0]

                def attn_v(kbi):
                    kb = border[kbi]
                    pt = p_ts[kb]
                    for dvc in range(4):
                        nc.tensor.matmul(
                            outT[dvc][:], vt[:, kb, dvc * P:(dvc + 1) * P], pt[:],
                            start=(kbi == 0), stop=(kbi == nkb - 1),
                        )

                border = list(range(nkb))
                for kbi, kb in enumerate(border):
                    s_ps = psB.tile(
                        [P, 512], F32, name=f"s{u}_{kb}", tag="sc", bufs=3
                    )
                    for dkc in range(4):
                        nc.tensor.matmul(
                            s_ps[:], kt[:, dkc, kb * P:(kb + 1) * P], qt[:, dkc, :],
                            start=(dkc == 0), stop=(dkc == 3),
                        )
                    pt = wp.tile([P, 512], FP16, name=f"p{u}_{kb}", tag="pt", bufs=6)
                    p_ts[kb] = pt
                    kb_l = kb - (nkb - 8)
                    if kb_l >= 0:
                        # halves: the DVE mask-mul on half 0 runs under the
                        # ScalarE exp of half 1
                        nc.scalar.activation(pt[:, 0:256], s_ps[:, 0:256],
                                             AF.Exp, scale=SCALE)
                        nc.scalar.activation(pt[:, 256:512], s_ps[:, 256:512],
                                             AF.Exp, scale=SCALE)
                        nc.vector.tensor_mul(pt[:, 0:256], pt[:, 0:256],
                                             mask[:, kb_l, 0:256])
                        nc.vector.tensor_mul(pt[:, 256:512], pt[:, 256:512],
                                             mask[:, kb_l, 256:512])
                    else:
                        nc.scalar.activation(pt[:], s_ps[:], AF.Exp, scale=SCALE)
                    # pre-sum probability tiles in groups of 4 for the l matmul
                    if kbi % 4 == 1:
                        acc = wp.tile([P, 512], FP16, name=f"ac{u}_{kbi//4}",
                                      tag="acc", bufs=2)
                        accs.append(acc)
                        nc.vector.tensor_add(acc[:], p_ts[border[kbi - 1]][:],
                                             pt[:])
                    elif kbi % 4 != 0:
                        nc.vector.tensor_add(accs[kbi // 4][:],
                                             accs[kbi // 4][:], pt[:])
                        if kbi % 4 == 3:
                            tree.append((0, accs[kbi // 4]))
                            while len(tree) > 1 and tree[-1][0] == tree[-2][0]:
                                lv, b = tree.pop()
                                _, a = tree.pop()
                                t = wp.tile([P, 512], FP16,
                                            name=f"lt{u}_{treen[0]}",
                                            tag="ltree", bufs=5)
                                treen[0] += 1
                                nc.vector.tensor_add(t[:], a[:], b[:])
                                tree.append((lv + 1, t))
                    if kbi >= LAG:
                        attn_v(kbi - LAG)
                for kbi in range(nkb - LAG, nkb):
                    attn_v(kbi)
                while len(tree) > 1:
                    lv, b = tree.pop()
                    _, a = tree.pop()
                    t = wp.tile([P, 512], FP16, name=f"lt{u}_{treen[0]}",
                                tag="ltree", bufs=5)
                    treen[0] += 1
                    nc.vector.tensor_add(t[:], a[:], b[:])
                    tree.append((lv + 1, t))
                p_tot = tree[0][1]
                # l^T: queries land on partitions so the normalization scale
                # becomes a per-partition scalar fused into the y eviction
                lT_ps = psB.tile([P, 4], F32, name=f"lT{u}", tag="l", bufs=1)
                for ic in range(4):
                    nc.tensor.matmul(
                        lT_ps[:, ic:ic + 1],
                        p_tot[:, ic * P:(ic + 1) * P], ones[:, 0:1],
                        start=True, stop=True,
                    )
                recipT = wp.tile([P, 4], F32, name=f"rT{u}", tag="recipT",
                                 bufs=1)
                nc.vector.reciprocal(recipT[:], lT_ps[:])

                # PE: Q projection of u+1 covers the DVE normalization chain;
                # its PSUM evictions go first in the DVE queue so the
                # out-projection's PSUM slots free early.
                if u + 1 < NU:
                    qproj(u + 1, qts[u + 1])

                attn = [
                    wp.tile([P, 512], FP16, name=f"a{u}_{d}", tag=f"attn{d}", bufs=1)
                    for d in range(4)
                ]
                for dvc in range(4):
                    nc.vector.tensor_copy(attn[dvc][:], outT[dvc][:])

                ytags = ["l", "outT0", "outT1", "outT2", "outT3"]
                for ic in range(4):
                    y_s = wp.tile([P, DO], F32, name=f"y{u}_{ic}", tag="ys", bufs=4)
                    for doc in range(2):
                        y_ps = psB.tile(
                            [P, 512], F32, name=f"yp{u}_{ic}_{doc}",
                            tag=ytags[(ic * 2 + doc) % 5], bufs=1,
                        )
                        for dvc in range(4):
                            nc.tensor.matmul(
                                y_ps[:],
                                attn[dvc][:, ic * P:(ic + 1) * P],
                                wout[:, dvc, doc * 512:(doc + 1) * 512],
                                start=(dvc == 0), stop=(dvc == 3),
                            )
                        # normalization on ScalarE (per-partition scale),
                        # bias add on DVE: two engines pipeline the eviction
                        y_t = wp.tile([P, 512], F32, name=f"yt{u}_{ic}_{doc}",
                                      tag="yt", bufs=4)
                        nc.scalar.activation(y_t[:], y_ps[:], AF.Copy,
                                             scale=recipT[:, ic:ic + 1])
                        nc.vector.tensor_add(
                            y_s[:, doc * 512:(doc + 1) * 512], y_t[:],
                            bout[:, doc * 512:(doc + 1) * 512],
                        )
                    y_dst = y_d.ap().rearrange("(a p) n -> p a n", p=P)
                    nc.sync.dma_start(
                        y_dst[:, u * 4 + ic, 0:512], y_s[:, 0:512],
                    )
                    # last u: keep the gpsimd queue empty so its final drain
                    # doesn't straggle behind the sync queue
                    q2 = nc.sync if u == NU - 1 else nc.gpsimd
                    q2.dma_start(
                        y_dst[:, u * 4 + ic, 512:1024], y_s[:, 512:1024],
                    )
            psB_cm.__exit__(None, None, None)
            pb.__exit__(None, None, None)

    nc.compile()
    return nc


def _get_nc():
    global _COMPILED
    if _COMPILED is None:
        _COMPILED = _build()
    return _COMPILED


def _make_mask(h: int) -> np.ndarray:
    # mask[p_j, kb_l, i_l] = 1 if key (kb_l*128 + p_j) - i_l <= h*512 else 0
    pj = np.arange(P)[:, None, None]
    kb_l = np.arange(8)[None, :, None]
    il = np.arange(512)[None, None, :]
    return ((kb_l * P + pj - il) <= h * 512).astype(np.float16)


def _pack_pT(m: np.ndarray, inner: int) -> np.ndarray:
    """[DIN?, C] -> [128, DIN//128, C] partition-major fp16 pack."""
    d0, c = m.shape
    return np.ascontiguousarray(
        m.reshape(d0 // P, P, c).transpose(1, 0, 2)
    ).astype(np.float16)


def _prep_inputs(x, w_qkv, w_out, b_out):
    wq = _pack_pT(np.ascontiguousarray(w_qkv[:, 0:DI]), DI)
    wk = _pack_pT(np.ascontiguousarray(w_qkv[:, DI:2 * DI]), DI)
    wv = _pack_pT(np.ascontiguousarray(w_qkv[:, 2 * DI:3 * DI]), DI)
    wout = _pack_pT(np.ascontiguousarray(w_out), DO)
    bout = np.broadcast_to(b_out.astype(np.float32), (P, DO)).copy()
    masks = [_make_mask(h) for h in range(2)]
    in_maps = []
    for c in range(NCORES):
        b, h = c // 2, c % 2
        xT = x[b].T  # [DIN, N]
        # xt[p, js, a, j] = xT[a*128+p, js*512+j] : 8KB contiguous per partition
        xt = np.ascontiguousarray(
            xT.reshape(8, P, NJS, 512).transpose(1, 2, 0, 3)
        ).astype(np.float16)
        qrows = np.concatenate(
            [np.arange((2 * u + h) * 512, (2 * u + h + 1) * 512) for u in range(NU)]
        )
        xqT = x[b][qrows].T  # [DIN, NQ]
        xqt = np.ascontiguousarray(
            xqT.reshape(8, P, NU, 512).transpose(1, 2, 0, 3)
        ).astype(np.float16)
        in_maps.append(
            dict(xt=xt, xqt=xqt, wq=wq, wk=wk, wv=wv,
                 wout=wout, bout=bout, mask=masks[h])
        )
    return in_maps


def _assemble(results):
    out = np.empty((B, N, DO), dtype=np.float32)
    for c in range(NCORES):
        b, h = c // 2, c % 2
        y = results[c]["y"]
        for u in range(NU):
            g = 2 * u + h
            out[b, g * 512:(g + 1) * 512, :] = y[u * 512:(u + 1) * 512, :]
    return out


def _run(inputs, **kw):
    nc = _get_nc()
    in_maps = _prep_inputs(
        np.asarray(inputs["x"], dtype=np.float32),
        np.asarray(inputs["w_qkv"], dtype=np.float32),
        np.asarray(inputs["w_out"], dtype=np.float32),
        np.asarray(inputs["b_out"], dtype=np.float32),
    )
    res = run_bass_kernel_spmd(nc, in_maps, list(range(NCORES)), **kw)
    return _assemble(res.results), res


def kernel(x, w_qkv, w_out, b_out):
    out, _ = _run(dict(x=x, w_qkv=w_qkv, w_out=w_out, b_out=b_out))
    return out


# revision 21
# speedup vs baseline: 1.0184x; 1.0184x over previous
"""Causal single-head attention (B=4, N=4096, d_in=1024, d_inner=512, d_out=1024)
for 8 Trainium2 NeuronCores.

Sharding: core c -> (batch b=c//2, half h=c%2). Each core handles the 4
global 512-row query blocks {2u+h : u=0..3} of batch b (block-interleaved
for causal load balance) and computes the full K/V projection on-chip.
No collectives; causality enters only through a per-core 0/1 mask input,
so the instruction stream is SPMD-uniform.

All matmul inputs are fp16 (fp32r/bf16 stream slower on this part); PSUM
accumulation stays fp32. Layouts are fully transposed (scores [j,i],
attention output [dv,i]) so the kernel contains no transposes. Softmax is
normalized AFTER the output projection: probability tiles are pre-summed
pairwise on the DVE, reduced to l^T [queries-on-partitions, 1] by tiny
free-dim-1 matmuls, and 1/l is applied as a per-partition ScalarE scale
during the y eviction (with the bias added by the DVE) - the lane-serial
DVE reciprocal never touches more than 4 elements per lane.
"""

import sys

if "/opt/trn_rl_repo" not in sys.path:
    sys.path.insert(0, "/opt/trn_rl_repo")

import numpy as np

import concourse.bacc as bacc
import concourse.mybir as mybir
import concourse.tile as tile
from concourse.bass_utils import run_bass_kernel_spmd

P = 128
B, N, DIN, DI, DO = 4, 4096, 1024, 512, 1024
NCORES = 8
NQ = N // 2          # query rows per core (2048)
NU = 4               # query super-blocks of 512 per core
NJS = N // 512       # key strips of 512 (8)
NKB = N // P         # key blocks of 128 (32)
SCALE = float(DI) ** -0.5

F32 = mybir.dt.float32
BF16 = mybir.dt.bfloat16
FP16 = mybir.dt.float16
AF = mybir.ActivationFunctionType

_COMPILED = None


def _build():
    nc = bacc.Bacc(None, target_bir_lowering=False)

    # host-packed layouts: per-partition-contiguous inner blocks
    xt_d = nc.dram_tensor("xt", [P, NJS, 8, 512], FP16, kind="ExternalInput")
    xqt_d = nc.dram_tensor("xqt", [P, NU, 8, 512], FP16, kind="ExternalInput")
    wq_d = nc.dram_tensor("wq", [P, 8, DI], FP16, kind="ExternalInput")
    wk_d = nc.dram_tensor("wk", [P, 8, DI], FP16, kind="ExternalInput")
    wv_d = nc.dram_tensor("wv", [P, 8, DI], FP16, kind="ExternalInput")
    wout_d = nc.dram_tensor("wout", [P, 4, DO], FP16, kind="ExternalInput")
    bout_d = nc.dram_tensor("bout", [P, DO], F32, kind="ExternalInput")
    mask_d = nc.dram_tensor("mask", [P, 8, 512], FP16, kind="ExternalInput")
    y_d = nc.dram_tensor("y", [NQ, DO], F32, kind="ExternalOutput")

    with tile.TileContext(nc) as tc:
        with tc.tile_pool(name="persist", bufs=1) as pp:
            kt = pp.tile([P, 4, N], FP16)        # K^T  [dk, j]   32KB/part
            vt = pp.tile([P, NKB, DI], FP16)     # V    [j, dv]   32KB/part
            wq = pp.tile([P, 8, DI], FP16)
            bout = pp.tile([P, DO], F32)
            ones = pp.tile([P, P], FP16)
            xqs_t = [pp.tile([P, 8, 512], FP16, name=f"xq{u}", tag="xqs", bufs=1)
                     for u in range(NU)]
            qts = [pp.tile([P, 4, 512], FP16, name=f"qt{u}", tag="qt", bufs=2)
                   for u in range(NU)]
            qts0 = qts[0]

            def qproj(u, qt, pool=None, tag="sc", bufs=3):
                """Q^T projection for super-block u into qt [P, 4, 512]."""
                for dq in range(4):
                    qps = (pool or psB).tile([P, 512], F32, name=f"q{u}_{dq}",
                                             tag=tag, bufs=bufs)
                    for c in range(8):
                        nc.tensor.matmul(
                            qps[:], wq[:, c, dq * P:(dq + 1) * P], xqs_t[u][:, c, :],
                            start=(c == 0), stop=(c == 7),
                        )
                    if pool is not None and dq % 2:
                        # u=0 runs at the PSUM pool handoff: alternate the
                        # eviction engine so the last one lands sooner
                        nc.scalar.activation(qt[:, dq, :], qps[:], AF.Copy)
                    else:
                        nc.vector.tensor_copy(qt[:, dq, :], qps[:])

            # ---- Phase A: K^T and V projection over all N keys ----
            # DMA order: first-strip deps land first; phase-B inputs trickle in.
            pa = tc.tile_pool(name="phaseA", bufs=1)
            wp = pa.__enter__()
            psKV_cm = tc.tile_pool(name="psKV", bufs=1, space="PSUM")
            psKV = psKV_cm.__enter__()
            wk = wp.tile([P, 8, DI], FP16)
            wv = wp.tile([P, 8, DI], FP16)
            xs0 = wp.tile([P, 8, 512], FP16, name="xs0", tag="xs", bufs=4)
            # one big DMA per operand half: each dma_start costs ~600ns of
            # queue-engine issue time, so few large transfers beat many small
            nc.sync.dma_start(wk[:, 0:2, :], wk_d.ap()[:, 0:2, :])
            nc.sync.dma_start(wk[:, 2:8, :], wk_d.ap()[:, 2:8, :])
            nc.gpsimd.dma_start(xs0[:, 0:2, :], xt_d.ap()[:, 0, 0:2, :])
            nc.gpsimd.dma_start(xs0[:, 2:8, :], xt_d.ap()[:, 0, 2:8, :])
            nc.gpsimd.dma_start(wv[:], wv_d.ap())
            late_dmas = {
                2: lambda: nc.sync.dma_start(xqs_t[0][:], xqt_d.ap()[:, 0]),
                4: lambda: nc.sync.dma_start(wq[:], wq_d.ap()),
                5: lambda: nc.sync.dma_start(mask_t[:, 0:4, :],
                                             mask_d.ap()[:, 0:4, :]),
                6: lambda: (nc.sync.dma_start(bout[:], bout_d.ap()),
                            nc.sync.dma_start(mask_t[:, 4:8, :],
                                              mask_d.ap()[:, 4:8, :])),
                7: lambda: nc.sync.dma_start(wout_t[:], wout_d.ap()),
            }
            nc.vector.memset(ones[:], 1.0)
            mask_t = pp.tile([P, 8, 512], FP16, name="mask")
            wout_t = pp.tile([P, 4, DO], FP16, name="wout")
            for js in range(NJS):
                if js == 0:
                    xs = xs0
                else:
                    xs = wp.tile([P, 8, 512], FP16, name=f"xs{js}", tag="xs", bufs=4)
                    nc.sync.dma_start(xs[:, 0:4, :], xt_d.ap()[:, js, 0:4, :])
                    nc.gpsimd.dma_start(xs[:, 4:8, :], xt_d.ap()[:, js, 4:8, :])
                if js in late_dmas:
                    late_dmas[js]()
                for dk in range(4):
                    kps = psKV.tile([P, 512], F32, name=f"k{js}_{dk}", tag="kv", bufs=8)
                    for c in range(8):
                        nc.tensor.matmul(
                            kps[:], wk[:, c, dk * P:(dk + 1) * P], xs[:, c, :],
                            start=(c == 0), stop=(c == 7),
                        )
                    nc.vector.tensor_copy(kt[:, dk, js * 512:(js + 1) * 512], kps[:])
                for jsub in range(4):
                    vps = psKV.tile([P, 512], F32, name=f"v{js}_{jsub}", tag="kv", bufs=8)
                    for c in range(8):
                        nc.tensor.matmul(
                            vps[:], xs[:, c, jsub * P:(jsub + 1) * P], wv[:, c, :],
                            start=(c == 0), stop=(c == 7),
                        )
                    nc.vector.tensor_copy(vt[:, js * 4 + jsub, :], vps[:])

            qproj(0, qts0, pool=psKV, tag="kv", bufs=8)
            psKV_cm.__exit__(None, None, None)
            pa.__exit__(None, None, None)

            # ---- Phase B: per query super-block u ----
            pb = tc.tile_pool(name="phaseB", bufs=1)
            wp = pb.__enter__()
            psB_cm = tc.tile_pool(name="psB", bufs=1, space="PSUM")
            psB = psB_cm.__enter__()
            mask = mask_t
            wout = wout_t


            LAG = 2
            for u in range(NU):
                nkb = 8 * u + 8  # key blocks (128) this super-block attends to
                ng = nkb // 4    # l-sum groups of 4 key blocks
                qt = qts[u]
                if u + 1 < NU:
                    nc.gpsimd.dma_start(xqs_t[u + 1][:], xqt_d.ap()[:, u + 1])

                outT = [
                    psB.tile([P, 512], F32, name=f"o{u}_{d}", tag=f"outT{d}", bufs=1)
                    for d in range(4)
                ]
                p_ts = {}
                accs = []
                tree = []  # (level, tile) stack for pairwise fp16 l summation
                treen = [0]

                def attn_v(kbi):
                    kb = border[kbi]
                    pt = p_ts[kb]
                    for dvc in range(4):
                        nc.tensor.matmul(
                            outT[dvc][:], vt[:, kb, dvc * P:(dvc + 1) * P], pt[:],
                            start=(kbi == 0), stop=(kbi == nkb - 1),
                        )

                border = list(range(nkb))
                for kbi, kb in enumerate(border):
                    s_ps = psB.tile(
                        [P, 512], F32, name=f"s{u}_{kb}", tag="sc", bufs=3
                    )
                    for dkc in range(4):
                        nc.tensor.matmul(
                            s_ps[:], kt[:, dkc, kb * P:(kb + 1) * P], qt[:, dkc, :],
                            start=(dkc == 0), stop=(dkc == 3),
                        )
                    pt = wp.tile([P, 512], FP16, name=f"p{u}_{kb}", tag="pt", bufs=6)
                    p_ts[kb] = pt
                    kb_l = kb - (nkb - 8)
                    if kb_l >= 0:
                        # halves: the DVE mask-mul on half 0 runs under the
                        # ScalarE exp of half 1
                        nc.scalar.activation(pt[:, 0:256], s_ps[:, 0:256],
                                             AF.Exp, scale=SCALE)
                        nc.scalar.activation(pt[:, 256:512], s_ps[:, 256:512],
                                             AF.Exp, scale=SCALE)
                        nc.vector.tensor_mul(pt[:, 0:256], pt[:, 0:256],
                                             mask[:, kb_l, 0:256])
                        nc.vector.tensor_mul(pt[:, 256:512], pt[:, 256:512],
                                             mask[:, kb_l, 256:512])
                    else:
                        nc.scalar.activation(pt[:], s_ps[:], AF.Exp, scale=SCALE)
                    # pre-sum probability tiles in groups of 4 for the l matmul
                    if kbi % 4 == 1:
                        acc = wp.tile([P, 512], FP16, name=f"ac{u}_{kbi//4}",
                                      tag="acc", bufs=2)
                        accs.append(acc)
                        nc.vector.tensor_add(acc[:], p_ts[border[kbi - 1]][:],
                                             pt[:])
                    elif kbi % 4 != 0:
                        nc.vector.tensor_add(accs[kbi // 4][:],
                                             accs[kbi // 4][:], pt[:])
                        if kbi % 4 == 3:
                            tree.append((0, accs[kbi // 4]))
                            while len(tree) > 1 and tree[-1][0] == tree[-2][0]:
                                lv, b = tree.pop()
                                _, a = tree.pop()
                                t = wp.tile([P, 512], FP16,
                                            name=f"lt{u}_{treen[0]}",
                                            tag="ltree", bufs=5)
                                treen[0] += 1
                                nc.vector.tensor_add(t[:], a[:], b[:])
                                tree.append((lv + 1, t))
                    if kbi >= LAG:
                        attn_v(kbi - LAG)
                for kbi in range(nkb - LAG, nkb):
                    attn_v(kbi)
                while len(tree) > 1:
                    lv, b = tree.pop()
                    _, a = tree.pop()
                    t = wp.tile([P, 512], FP16, name=f"lt{u}_{treen[0]}",
                                tag="ltree", bufs=5)
                    treen[0] += 1
                    nc.vector.tensor_add(t[:], a[:], b[:])
                    tree.append((lv + 1, t))
                p_tot = tree[0][1]
                # l^T: queries land on partitions so the normalization scale
                # becomes a per-partition scalar fused into the y eviction
                lT_ps = psB.tile([P, 4], F32, name=f"lT{u}", tag="l", bufs=1)
                for ic in range(4):
                    nc.tensor.matmul(
                        lT_ps[:, ic:ic + 1],
                        p_tot[:, ic * P:(ic + 1) * P], ones[:, 0:1],
                        start=True, stop=True,
                    )
                recipT = wp.tile([P, 4], F32, name=f"rT{u}", tag="recipT",
                                 bufs=1)
                nc.vector.reciprocal(recipT[:], lT_ps[:])

                # PE: Q projection of u+1 covers the DVE normalization chain;
                # its PSUM evictions go first in the DVE queue so the
                # out-projection's PSUM slots free early.
                if u + 1 < NU:
                    qproj(u + 1, qts[u + 1])

                attn = [
                    wp.tile([P, 512], FP16, name=f"a{u}_{d}", tag=f"attn{d}", bufs=1)
                    for d in range(4)
                ]
                for dvc in range(4):
                    nc.vector.tensor_copy(attn[dvc][:], outT[dvc][:])

                ytags = ["l", "outT0", "outT1", "outT2", "outT3"]
                for ic in range(4):
                    y_s = wp.tile([P, DO], F32, name=f"y{u}_{ic}", tag="ys", bufs=4)
                    for doc in range(2):
                        y_ps = psB.tile(
                            [P, 512], F32, name=f"yp{u}_{ic}_{doc}",
                            tag=ytags[(ic * 2 + doc) % 5], bufs=1,
                        )
                        for dvc in range(4):
                            nc.tensor.matmul(
                                y_ps[:],
                                attn[dvc][:, ic * P:(ic + 1) * P],
                                wout[:, dvc, doc * 512:(doc + 1) * 512],
                                start=(dvc == 0), stop=(dvc == 3),
                            )
                        # normalization on ScalarE (per-partition scale),
                        # bias add on DVE: two engines pipeline the eviction
                        y_t = wp.tile([P, 512], F32, name=f"yt{u}_{ic}_{doc}",
                                      tag="yt", bufs=4)
                        nc.scalar.activation(y_t[:], y_ps[:], AF.Copy,
                                             scale=recipT[:, ic:ic + 1])
                        nc.vector.tensor_add(
                            y_s[:, doc * 512:(doc + 1) * 512], y_t[:],
                            bout[:, doc * 512:(doc + 1) * 512],
                        )
                    y_dst = y_d.ap().rearrange("(a p) n -> p a n", p=P)
                    nc.sync.dma_start(
                        y_dst[:, u * 4 + ic, 0:512], y_s[:, 0:512],
                    )
                    # last u: keep the gpsimd queue empty so its final drain
                    # doesn't straggle behind the sync queue
                    q2 = nc.sync if u == NU - 1 else nc.gpsimd
                    q2.dma_start(
                        y_dst[:, u * 4 + ic, 512:1024], y_s[:, 512:1024],
                    )
            psB_cm.__exit__(None, None, None)
            pb.__exit__(None, None, None)

    nc.compile()
    return nc


def _get_nc():
    global _COMPILED
    if _COMPILED is None:
        _COMPILED = _build()
    return _COMPILED


def _make_mask(h: int) -> np.ndarray:
    # mask[p_j, kb_l, i_l] = 1 if key (kb_l*128 + p_j) - i_l <= h*512 else 0
    pj = np.arange(P)[:, None, None]
    kb_l = np.arange(8)[None, :, None]
    il = np.arange(512)[None, None, :]
    return ((kb_l * P + pj - il) <= h * 512).astype(np.float16)


def _pack_pT(m: np.ndarray, inner: int) -> np.ndarray:
    """[DIN?, C] -> [128, DIN//128, C] partition-major fp16 pack."""
    d0, c = m.shape
    return np.ascontiguousarray(
        m.reshape(d0 // P, P, c).transpose(1, 0, 2)
    ).astype(np.float16)


def _prep_inputs(x, w_qkv, w_out, b_out):
    wq = _pack_pT(np.ascontiguousarray(w_qkv[:, 0:DI]), DI)
    wk = _pack_pT(np.ascontiguousarray(w_qkv[:, DI:2 * DI]), DI)
    wv = _pack_pT(np.ascontiguousarray(w_qkv[:, 2 * DI:3 * DI]), DI)
    wout = _pack_pT(np.ascontiguousarray(w_out), DO)
    bout = np.broadcast_to(b_out.astype(np.float32), (P, DO)).copy()
    masks = [_make_mask(h) for h in range(2)]
    in_maps = []
    for c in range(NCORES):
        b, h = c // 2, c % 2
        xT = x[b].T  # [DIN, N]
        # xt[p, js, a, j] = xT[a*128+p, js*512+j] : 8KB contiguous per partition
        xt = np.ascontiguousarray(
            xT.reshape(8, P, NJS, 512).transpose(1, 2, 0, 3)
        ).astype(np.float16)
        qrows = np.concatenate(
            [np.arange((2 * u + h) * 512, (2 * u + h + 1) * 512) for u in range(NU)]
        )
        xqT = x[b][qrows].T  # [DIN, NQ]
        xqt = np.ascontiguousarray(
            xqT.reshape(8, P, NU, 512).transpose(1, 2, 0, 3)
        ).astype(np.float16)
        in_maps.append(
            dict(xt=xt, xqt=xqt, wq=wq, wk=wk, wv=wv,
                 wout=wout, bout=bout, mask=masks[h])
        )
    return in_maps


def _assemble(results):
    out = np.empty((B, N, DO), dtype=np.float32)
    for c in range(NCORES):
        b, h = c // 2, c % 2
        y = results[c]["y"]
        for u in range(NU):
            g = 2 * u + h
            out[b, g * 512:(g + 1) * 512, :] = y[u * 512:(u + 1) * 512, :]
    return out


def _run(inputs, **kw):
    nc = _get_nc()
    in_maps = _prep_inputs(
        np.asarray(inputs["x"], dtype=np.float32),
        np.asarray(inputs["w_qkv"], dtype=np.float32),
        np.asarray(inputs["w_out"], dtype=np.float32),
        np.asarray(inputs["b_out"], dtype=np.float32),
    )
    res = run_bass_kernel_spmd(nc, in_maps, list(range(NCORES)), **kw)
    return _assemble(res.results), res


def kernel(x, w_qkv, w_out, b_out):
    out, _ = _run(dict(x=x, w_qkv=w_qkv, w_out=w_out, b_out=b_out))
    return out


# revision 22
# speedup vs baseline: 1.0203x; 1.0019x over previous
"""Causal single-head attention (B=4, N=4096, d_in=1024, d_inner=512, d_out=1024)
for 8 Trainium2 NeuronCores.

Sharding: core c -> (batch b=c//2, half h=c%2). Each core handles the 4
global 512-row query blocks {2u+h : u=0..3} of batch b (block-interleaved
for causal load balance) and computes the full K/V projection on-chip.
No collectives; causality enters only through a per-core 0/1 mask input,
so the instruction stream is SPMD-uniform.

All matmul inputs are fp16 (fp32r/bf16 stream slower on this part); PSUM
accumulation stays fp32. Layouts are fully transposed (scores [j,i],
attention output [dv,i]) so the kernel contains no transposes. Softmax is
normalized AFTER the output projection: probability tiles are pre-summed
pairwise on the DVE, reduced to l^T [queries-on-partitions, 1] by tiny
free-dim-1 matmuls, and 1/l is applied as a per-partition ScalarE scale
during the y eviction (with the bias added by the DVE) - the lane-serial
DVE reciprocal never touches more than 4 elements per lane.
"""

import sys

if "/opt/trn_rl_repo" not in sys.path:
    sys.path.insert(0, "/opt/trn_rl_repo")

import numpy as np

import concourse.bacc as bacc
import concourse.mybir as mybir
import concourse.tile as tile
from concourse.bass_utils import run_bass_kernel_spmd

P = 128
B, N, DIN, DI, DO = 4, 4096, 1024, 512, 1024
NCORES = 8
NQ = N // 2          # query rows per core (2048)
NU = 4               # query super-blocks of 512 per core
NJS = N // 512       # key strips of 512 (8)
NKB = N // P         # key blocks of 128 (32)
SCALE = float(DI) ** -0.5

F32 = mybir.dt.float32
BF16 = mybir.dt.bfloat16
FP16 = mybir.dt.float16
AF = mybir.ActivationFunctionType

_COMPILED = None


def _build():
    nc = bacc.Bacc(None, target_bir_lowering=False)

    # host-packed layouts: per-partition-contiguous inner blocks
    xt_d = nc.dram_tensor("xt", [P, NJS, 8, 512], FP16, kind="ExternalInput")
    xqt_d = nc.dram_tensor("xqt", [P, NU, 8, 512], FP16, kind="ExternalInput")
    wq_d = nc.dram_tensor("wq", [P, 8, DI], FP16, kind="ExternalInput")
    wk_d = nc.dram_tensor("wk", [P, 8, DI], FP16, kind="ExternalInput")
    wv_d = nc.dram_tensor("wv", [P, 8, DI], FP16, kind="ExternalInput")
    wout_d = nc.dram_tensor("wout", [P, 4, DO], FP16, kind="ExternalInput")
    bout_d = nc.dram_tensor("bout", [P, DO], F32, kind="ExternalInput")
    mask_d = nc.dram_tensor("mask", [P, 8, 512], FP16, kind="ExternalInput")
    y_d = nc.dram_tensor("y", [NQ, DO], F32, kind="ExternalOutput")

    with tile.TileContext(nc) as tc:
        with tc.tile_pool(name="persist", bufs=1) as pp:
            kt = pp.tile([P, 4, N], FP16)        # K^T  [dk, j]   32KB/part
            vt = pp.tile([P, NKB, DI], FP16)     # V    [j, dv]   32KB/part
            wq = pp.tile([P, 8, DI], FP16)
            bout = pp.tile([P, DO], F32)
            ones = pp.tile([P, P], FP16)
            xqs_t = [pp.tile([P, 8, 512], FP16, name=f"xq{u}", tag="xqs", bufs=1)
                     for u in range(NU)]
            qts = [pp.tile([P, 4, 512], FP16, name=f"qt{u}", tag="qt", bufs=2)
                   for u in range(NU)]
            qts0 = qts[0]

            def qproj(u, qt, pool=None, tag="sc", bufs=3):
                """Q^T projection for super-block u into qt [P, 4, 512]."""
                for dq in range(4):
                    qps = (pool or psB).tile([P, 512], F32, name=f"q{u}_{dq}",
                                             tag=tag, bufs=bufs)
                    for c in range(8):
                        nc.tensor.matmul(
                            qps[:], wq[:, c, dq * P:(dq + 1) * P], xqs_t[u][:, c, :],
                            start=(c == 0), stop=(c == 7),
                        )
                    if pool is not None and dq % 2:
                        # u=0 runs at the PSUM pool handoff: alternate the
                        # eviction engine so the last one lands sooner
                        nc.scalar.activation(qt[:, dq, :], qps[:], AF.Copy)
                    else:
                        nc.vector.tensor_copy(qt[:, dq, :], qps[:])

            # ---- Phase A: K^T and V projection over all N keys ----
            # DMA order: first-strip deps land first; phase-B inputs trickle in.
            pa = tc.tile_pool(name="phaseA", bufs=1)
            wp = pa.__enter__()
            psKV_cm = tc.tile_pool(name="psKV", bufs=1, space="PSUM")
            psKV = psKV_cm.__enter__()
            wk = wp.tile([P, 8, DI], FP16)
            wv = wp.tile([P, 8, DI], FP16)
            xs0 = wp.tile([P, 8, 512], FP16, name="xs0", tag="xs", bufs=4)
            # one big DMA per operand half: each dma_start costs ~600ns of
            # queue-engine issue time, so few large transfers beat many small
            nc.sync.dma_start(wk[:, 0:2, :], wk_d.ap()[:, 0:2, :])
            nc.sync.dma_start(wk[:, 2:8, :], wk_d.ap()[:, 2:8, :])
            nc.gpsimd.dma_start(xs0[:, 0:2, :], xt_d.ap()[:, 0, 0:2, :])
            nc.gpsimd.dma_start(xs0[:, 2:8, :], xt_d.ap()[:, 0, 2:8, :])
            nc.gpsimd.dma_start(wv[:], wv_d.ap())
            late_dmas = {
                2: lambda: nc.sync.dma_start(xqs_t[0][:], xqt_d.ap()[:, 0]),
                4: lambda: nc.sync.dma_start(wq[:], wq_d.ap()),
                5: lambda: nc.sync.dma_start(mask_t[:, 0:4, :],
                                             mask_d.ap()[:, 0:4, :]),
                6: lambda: (nc.sync.dma_start(bout[:], bout_d.ap()),
                            nc.sync.dma_start(mask_t[:, 4:8, :],
                                              mask_d.ap()[:, 4:8, :])),
                7: lambda: nc.sync.dma_start(wout_t[:], wout_d.ap()),
            }
            nc.vector.memset(ones[:], 1.0)
            mask_t = pp.tile([P, 8, 512], FP16, name="mask")
            wout_t = pp.tile([P, 4, DO], FP16, name="wout")
            for js in range(NJS):
                if js == 0:
                    xs = xs0
                else:
                    xs = wp.tile([P, 8, 512], FP16, name=f"xs{js}", tag="xs", bufs=4)
                    nc.sync.dma_start(xs[:, 0:4, :], xt_d.ap()[:, js, 0:4, :])
                    nc.gpsimd.dma_start(xs[:, 4:8, :], xt_d.ap()[:, js, 4:8, :])
                if js in late_dmas:
                    late_dmas[js]()
                for dk in range(4):
                    kps = psKV.tile([P, 512], F32, name=f"k{js}_{dk}", tag="kv", bufs=8)
                    for c in range(8):
                        nc.tensor.matmul(
                            kps[:], wk[:, c, dk * P:(dk + 1) * P], xs[:, c, :],
                            start=(c == 0), stop=(c == 7),
                        )
                    nc.vector.tensor_copy(kt[:, dk, js * 512:(js + 1) * 512], kps[:])
                for jsub in range(4):
                    vps = psKV.tile([P, 512], F32, name=f"v{js}_{jsub}", tag="kv", bufs=8)
                    for c in range(8):
                        nc.tensor.matmul(
                            vps[:], xs[:, c, jsub * P:(jsub + 1) * P], wv[:, c, :],
                            start=(c == 0), stop=(c == 7),
                        )
                    nc.vector.tensor_copy(vt[:, js * 4 + jsub, :], vps[:])

            qproj(0, qts0, pool=psKV, tag="kv", bufs=8)
            psKV_cm.__exit__(None, None, None)
            pa.__exit__(None, None, None)

            # ---- Phase B: per query super-block u ----
            pb = tc.tile_pool(name="phaseB", bufs=1)
            wp = pb.__enter__()
            psB_cm = tc.tile_pool(name="psB", bufs=1, space="PSUM")
            psB = psB_cm.__enter__()
            mask = mask_t
            wout = wout_t


            LAG = 2
            for u in range(NU):
                nkb = 8 * u + 8  # key blocks (128) this super-block attends to
                qt = qts[u]
                if u + 1 < NU:
                    nc.gpsimd.dma_start(xqs_t[u + 1][:], xqt_d.ap()[:, u + 1])

                outT = [
                    psB.tile([P, 512], F32, name=f"o{u}_{d}", tag=f"outT{d}", bufs=1)
                    for d in range(4)
                ]
                p_ts = {}
                accs = []
                tree = []  # (level, tile) stack for pairwise fp16 l summation
                treen = [0]

                def attn_v(kbi):
                    kb = border[kbi]
                    pt = p_ts[kb]
                    for dvc in range(4):
                        nc.tensor.matmul(
                            outT[dvc][:], vt[:, kb, dvc * P:(dvc + 1) * P], pt[:],
                            start=(kbi == 0), stop=(kbi == nkb - 1),
                        )

                border = list(range(nkb))
                for kbi, kb in enumerate(border):
                    s_ps = psB.tile(
                        [P, 512], F32, name=f"s{u}_{kb}", tag="sc", bufs=3
                    )
                    for dkc in range(4):
                        nc.tensor.matmul(
                            s_ps[:], kt[:, dkc, kb * P:(kb + 1) * P], qt[:, dkc, :],
                            start=(dkc == 0), stop=(dkc == 3),
                        )
                    pt = wp.tile([P, 512], FP16, name=f"p{u}_{kb}", tag="pt", bufs=6)
                    p_ts[kb] = pt
                    kb_l = kb - (nkb - 8)
                    if kb_l >= 0:
                        # halves: the DVE mask-mul on half 0 runs under the
                        # ScalarE exp of half 1
                        nc.scalar.activation(pt[:, 0:256], s_ps[:, 0:256],
                                             AF.Exp, scale=SCALE)
                        nc.scalar.activation(pt[:, 256:512], s_ps[:, 256:512],
                                             AF.Exp, scale=SCALE)
                        nc.vector.tensor_mul(pt[:, 0:256], pt[:, 0:256],
                                             mask[:, kb_l, 0:256])
                        nc.vector.tensor_mul(pt[:, 256:512], pt[:, 256:512],
                                             mask[:, kb_l, 256:512])
                    else:
                        nc.scalar.activation(pt[:], s_ps[:], AF.Exp, scale=SCALE)
                    # pre-sum probability tiles in groups of 4 for the l matmul
                    if kbi % 4 == 1:
                        acc = wp.tile([P, 512], FP16, name=f"ac{u}_{kbi//4}",
                                      tag="acc", bufs=2)
                        accs.append(acc)
                        nc.vector.tensor_add(acc[:], p_ts[border[kbi - 1]][:],
                                             pt[:])
                    elif kbi % 4 != 0:
                        nc.vector.tensor_add(accs[kbi // 4][:],
                                             accs[kbi // 4][:], pt[:])
                        if kbi % 4 == 3:
                            tree.append((0, accs[kbi // 4]))
                            while len(tree) > 1 and tree[-1][0] == tree[-2][0]:
                                lv, b = tree.pop()
                                _, a = tree.pop()
                                t = wp.tile([P, 512], FP16,
                                            name=f"lt{u}_{treen[0]}",
                                            tag="ltree", bufs=5)
                                treen[0] += 1
                                nc.vector.tensor_add(t[:], a[:], b[:])
                                tree.append((lv + 1, t))
                    if kbi >= LAG:
                        attn_v(kbi - LAG)
                for kbi in range(nkb - LAG, nkb):
                    attn_v(kbi)
                while len(tree) > 1:
                    lv, b = tree.pop()
                    _, a = tree.pop()
                    t = wp.tile([P, 512], FP16, name=f"lt{u}_{treen[0]}",
                                tag="ltree", bufs=5)
                    treen[0] += 1
                    nc.vector.tensor_add(t[:], a[:], b[:])
                    tree.append((lv + 1, t))
                p_tot = tree[0][1]
                # l^T: queries land on partitions so the normalization scale
                # becomes a per-partition scalar fused into the y eviction
                lT_ps = psB.tile([P, 4], F32, name=f"lT{u}", tag="l", bufs=1)
                for ic in range(4):
                    nc.tensor.matmul(
                        lT_ps[:, ic:ic + 1],
                        p_tot[:, ic * P:(ic + 1) * P], ones[:, 0:1],
                        start=True, stop=True,
                    )
                recipT = wp.tile([P, 4], F32, name=f"rT{u}", tag="recipT",
                                 bufs=1)
                nc.vector.reciprocal(recipT[:], lT_ps[:])

                # PE: Q projection of u+1 covers the DVE normalization chain;
                # its PSUM evictions go first in the DVE queue so the
                # out-projection's PSUM slots free early.
                if u + 1 < NU:
                    qproj(u + 1, qts[u + 1])

                attn = [
                    wp.tile([P, 512], FP16, name=f"a{u}_{d}", tag=f"attn{d}", bufs=1)
                    for d in range(4)
                ]
                for dvc in range(4):
                    nc.vector.tensor_copy(attn[dvc][:], outT[dvc][:])

                ytags = ["l", "outT0", "outT1", "outT2", "outT3"]
                for ic in range(4):
                    y_s = wp.tile([P, DO], F32, name=f"y{u}_{ic}", tag="ys", bufs=4)
                    for doc in range(2):
                        y_ps = psB.tile(
                            [P, 512], F32, name=f"yp{u}_{ic}_{doc}",
                            tag=ytags[(ic * 2 + doc) % 5], bufs=1,
                        )
                        for dvc in range(4):
                            nc.tensor.matmul(
                                y_ps[:],
                                attn[dvc][:, ic * P:(ic + 1) * P],
                                wout[:, dvc, doc * 512:(doc + 1) * 512],
                                start=(dvc == 0), stop=(dvc == 3),
                            )
                        # normalization on ScalarE (per-partition scale),
                        # bias add on DVE: two engines pipeline the eviction
                        y_t = wp.tile([P, 512], F32, name=f"yt{u}_{ic}_{doc}",
                                      tag="yt", bufs=4)
                        nc.scalar.activation(y_t[:], y_ps[:], AF.Copy,
                                             scale=recipT[:, ic:ic + 1])
                        nc.vector.tensor_add(
                            y_s[:, doc * 512:(doc + 1) * 512], y_t[:],
                            bout[:, doc * 512:(doc + 1) * 512],
                        )
                    y_dst = y_d.ap().rearrange("(a p) n -> p a n", p=P)
                    nc.sync.dma_start(
                        y_dst[:, u * 4 + ic, 0:512], y_s[:, 0:512],
                    )
                    # last u: keep the gpsimd queue empty so its final drain
                    # doesn't straggle behind the sync queue
                    q2 = nc.sync if u == NU - 1 else nc.gpsimd
                    q2.dma_start(
                        y_dst[:, u * 4 + ic, 512:1024], y_s[:, 512:1024],
                    )
            psB_cm.__exit__(None, None, None)
            pb.__exit__(None, None, None)

    nc.compile()
    return nc


def _get_nc():
    global _COMPILED
    if _COMPILED is None:
        _COMPILED = _build()
    return _COMPILED


def _make_mask(h: int) -> np.ndarray:
    # mask[p_j, kb_l, i_l] = 1 if key (kb_l*128 + p_j) - i_l <= h*512 else 0
    pj = np.arange(P)[:, None, None]
    kb_l = np.arange(8)[None, :, None]
    il = np.arange(512)[None, None, :]
    return ((kb_l * P + pj - il) <= h * 512).astype(np.float16)


def _pack_pT(m: np.ndarray, inner: int) -> np.ndarray:
    """[DIN?, C] -> [128, DIN//128, C] partition-major fp16 pack."""
    d0, c = m.shape
    return np.ascontiguousarray(
        m.reshape(d0 // P, P, c).transpose(1, 0, 2)
    ).astype(np.float16)


def _prep_inputs(x, w_qkv, w_out, b_out):
    wq = _pack_pT(np.ascontiguousarray(w_qkv[:, 0:DI]), DI)
    wk = _pack_pT(np.ascontiguousarray(w_qkv[:, DI:2 * DI]), DI)
    wv = _pack_pT(np.ascontiguousarray(w_qkv[:, 2 * DI:3 * DI]), DI)
    wout = _pack_pT(np.ascontiguousarray(w_out), DO)
    bout = np.broadcast_to(b_out.astype(np.float32), (P, DO)).copy()
    masks = [_make_mask(h) for h in range(2)]
    in_maps = []
    for c in range(NCORES):
        b, h = c // 2, c % 2
        xT = x[b].T  # [DIN, N]
        # xt[p, js, a, j] = xT[a*128+p, js*512+j] : 8KB contiguous per partition
        xt = np.ascontiguousarray(
            xT.reshape(8, P, NJS, 512).transpose(1, 2, 0, 3)
        ).astype(np.float16)
        qrows = np.concatenate(
            [np.arange((2 * u + h) * 512, (2 * u + h + 1) * 512) for u in range(NU)]
        )
        xqT = x[b][qrows].T  # [DIN, NQ]
        xqt = np.ascontiguousarray(
            xqT.reshape(8, P, NU, 512).transpose(1, 2, 0, 3)
        ).astype(np.float16)
        in_maps.append(
            dict(xt=xt, xqt=xqt, wq=wq, wk=wk, wv=wv,
                 wout=wout, bout=bout, mask=masks[h])
        )
    return in_maps


def _assemble(results):
    out = np.empty((B, N, DO), dtype=np.float32)
    for c in range(NCORES):
        b, h = c // 2, c % 2
        y = results[c]["y"]
        for u in range(NU):
            g = 2 * u + h
            out[b, g * 512:(g + 1) * 512, :] = y[u * 512:(u + 1) * 512, :]
    return out


def _run(inputs, **kw):
    nc = _get_nc()
    in_maps = _prep_inputs(
        np.asarray(inputs["x"], dtype=np.float32),
        np.asarray(inputs["w_qkv"], dtype=np.float32),
        np.asarray(inputs["w_out"], dtype=np.float32),
        np.asarray(inputs["b_out"], dtype=np.float32),
    )
    res = run_bass_kernel_spmd(nc, in_maps, list(range(NCORES)), **kw)
    return _assemble(res.results), res


def kernel(x, w_qkv, w_out, b_out):
    out, _ = _run(dict(x=x, w_qkv=w_qkv, w_out=w_out, b_out=b_out))
    return out
